# revision 24
# baseline (speedup 1.0000x reference)
"""Trainium2 Bass kernel for nn_ColPredictor (3x LSTM + dual attention heads).

Self-contained: hardcoded shapes, pure data-parallel over batch (B=128 -> 16/core
on 8 cores). Per core: fused-bias GEMM for LSTM input transforms (xg), a
tri-LSTM-interleaved recurrence (float32r matmuls, PE transposes for the hidden
state), then attention + output heads. Outputs are returned in host-friendly
layouts and reassembled in numpy.
"""

import os
import sys
import numpy as np
from contextlib import ExitStack

sys.path.insert(0, "/opt/trn_rl_repo")

import concourse.bass as bass
import concourse.tile as tile
from concourse import bacc, mybir
from concourse.bass_utils import run_bass_kernel_spmd

F32 = mybir.dt.float32
F32R = mybir.dt.float32r
AF = mybir.ActivationFunctionType
AX = mybir.AxisListType

B, Q, S, C, D, H = 128, 64, 32, 128, 300, 512
G = 4 * H          # 2048 gate width
NC = 8             # cores
BL = B // NC       # 16 batch per core
DP = 384           # padded contraction for x (300 data + bias row at 300)
NL = 4             # num labels
# gate reorder: torch (i,f,g,o) -> (i,f,o,g) so sigmoid gates are contiguous
PERM = np.r_[0:1024, 1536:2048, 1024:1536]

LSTMS = [("col", C), ("q", Q), ("hs", S)]   # descending seqlen; partitions 0/32/64
TMAX = C


def _r(x32r):
    return x32r.bitcast(F32R)


def build_program():
    nc = bacc.Bacc("TRN2", target_bir_lowering=False, debug=False, num_devices=NC)

    def inp(name, shape, dt=F32R):
        return nc.dram_tensor(name, shape, dt, kind="ExternalInput").ap()

    ins = {}
    for nm, T in LSTMS:
        ins[f"xt_{nm}"] = inp(f"xt_{nm}", [128, 3 * T * BL])
        ins[f"wih_{nm}"] = inp(f"wih_{nm}", [128, 3 * G])
        ins[f"whh_{nm}"] = inp(f"whh_{nm}", [128, 4 * G])
    ins["i16rep"] = inp("i16rep", [128, 16])       # eye(16) tiled 8x along partitions

    ins["ident"] = inp("ident", [128, 128])        # eye(128)
    ins["ones4"] = inp("ones4", [128, 4])
    ins["m1q"] = inp("m1q", [128, BL * Q], F32)         # outer(col_mask, q_mask) per b
    ins["m1s"] = inp("m1s", [128, BL * S], F32)
    for w in ["watt_qn", "watt_hn", "watt_q", "watt_h",
              "wout_q", "wout_hs", "wnum_q", "wnum_hs", "wout_c"]:
        ins[w] = inp(w, [128, 4 * H])
    for b_ in ["batt_qn", "batt_hn", "batt_q", "batt_h", "bout_sum", "bnum_sum"]:
        ins[b_] = inp(b_, [128, 4], F32)
    ins["wno"] = inp("wno", [128, 16])             # col_num_out.T chunks [128, kc*4+l]
    ins["bno"] = inp("bno", [4, 1], F32)
    ins["vout"] = inp("vout", [128, 4])            # col_out.T chunks
    ins["b0"] = inp("b0", [1, 1], F32)

    out_num = nc.dram_tensor("out_num", [NL, BL], F32, kind="ExternalOutput").ap()
    out_col = nc.dram_tensor("out_col", [1, BL * C], F32, kind="ExternalOutput").ap()

    # internal DRAM for xg and encodings (transposed layout)
    xg_dram = {nm: nc.dram_tensor(f"xg_{nm}", [T * BL, G], F32R).ap() for nm, T in LSTMS}
    encT_dram = {nm: nc.dram_tensor(f"encT_{nm}", [128, 4 * T * BL], F32R).ap()
                 for nm, T in LSTMS}

    with tile.TileContext(nc) as tc, ExitStack() as top:
        cpool = top.enter_context(tc.tile_pool(name="consts", bufs=1))

        def load_const(name, shape):
            t = cpool.tile(shape, F32R, tag=name, name=name)
            nc.sync.dma_start(t[:], ins[name])
            return t

        i16rep = load_const("i16rep", [128, 16])
        ident = load_const("ident", [128, 128])
        ones4 = load_const("ones4", [128, 4])

        # ---------------- Phase 1: xg = [x;1] @ [Wih; bias].T  ------------
        with tc.tile_pool(name="p1", bufs=2) as p1, \
             tc.tile_pool(name="p1psum", bufs=2, space=bass.MemorySpace.PSUM) as p1p, \
             tc.tile_pool(name="p1out", bufs=3) as p1o:
            for nm, T in LSTMS:
                ntok = T * BL
                wih = p1.tile([128, 3 * G], F32R, tag="wih")
                nc.sync.dma_start(wih[:], ins[f"wih_{nm}"])
                xt = p1.tile([128, 3 * ntok], F32R, tag="xt")
                nc.sync.dma_start(xt[:], ins[f"xt_{nm}"])
                for tc128 in range(ntok // 128):
                    for n4 in range(4):
                        ps = p1p.tile([128, 512], F32, tag="xgp")
                        for kc in range(3):
                            nc.tensor.matmul(
                                ps[:],
                                _r(xt[:, kc * ntok + tc128 * 128:
                                      kc * ntok + (tc128 + 1) * 128]),
                                _r(wih[:, kc * G + n4 * 512: kc * G + (n4 + 1) * 512]),
                                start=(kc == 0), stop=(kc == 2))
                        ev = p1o.tile([128, 512], F32R, tag="xgev")
                        nc.vector.tensor_copy(ev[:], ps[:])
                        nc.sync.dma_start(
                            xg_dram[nm][tc128 * 128:(tc128 + 1) * 128,
                                        n4 * 512:(n4 + 1) * 512], ev[:])

        # ---------------- Phase 2: recurrence ----------------------------
        with tc.tile_pool(name="whh", bufs=1) as whp, \
             tc.tile_pool(name="rec", bufs=1) as rec, \
             tc.tile_pool(name="xg", bufs=6) as xgp, \
             tc.tile_pool(name="ht", bufs=6) as htp, \
             tc.tile_pool(name="recps", bufs=1, space=bass.MemorySpace.PSUM) as rpp, \
             tc.tile_pool(name="trps", bufs=2, space=bass.MemorySpace.PSUM) as tpp, \
             tc.tile_pool(name="work", bufs=2) as wk:
            whh = {}
            for nm, T in LSTMS:
                whh[nm] = whp.tile([128, 4 * G], F32R, tag=f"whh_{nm}", name=f"whh_{nm}")
                nc.sync.dma_start(whh[nm][:], ins[f"whh_{nm}"])

            c_st = {}
            hT_st = {}
            for nm, _ in LSTMS:
                c_st[nm] = rec.tile([16, 512], F32, tag=f"c_{nm}", name=f"c_{nm}")
                nc.gpsimd.memset(c_st[nm][:], 0.0)
                hT_st[nm] = rec.tile([128, 64], F32R, tag=f"hTs_{nm}",
                                     name=f"hTs_{nm}")

            for t in range(TMAX):
                active = [(i, nm, T) for i, (nm, T) in enumerate(LSTMS) if t < T]
                for li, nm, T in active:
                    xg_sb = xgp.tile([16, G], F32R, tag="xg")
                    nc.sync.dma_start(xg_sb[:], xg_dram[nm][t * BL:(t + 1) * BL, :])
                    psum2 = rpp.tile([16, G], F32, tag="psum2")
                    for n4 in range(4):
                        dstw = psum2[0:16, n4 * 512:(n4 + 1) * 512]
                        nc.tensor.matmul(
                            dstw, _r(i16rep[0:16, :]),
                            _r(xg_sb[:, n4 * 512:(n4 + 1) * 512]),
                            start=True, stop=(t == 0))
                        if t > 0:
                            hv = hT_st[nm][:].rearrange("p (k b) -> p k b", k=4)
                            for kc in range(4):
                                nc.tensor.matmul(
                                    dstw, _r(hv[:, kc, :]),
                                    _r(whh[nm][:, kc * G + n4 * 512:
                                               kc * G + (n4 + 1) * 512]),
                                    start=False, stop=(kc == 3))
                    # nonlinearities (rows 0:16 are real, 16:32 harmless copies)
                    sifo = wk.tile([16, 1536], F32, tag="sifo")
                    nc.scalar.activation(sifo[:], psum2[0:16, 0:1536], AF.Sigmoid)
                    tg = wk.tile([16, 512], F32, tag="tg")
                    nc.scalar.activation(tg[:], psum2[0:16, 1536:2048], AF.Tanh)
                    tmp = wk.tile([16, 512], F32, tag="tmp")
                    nc.vector.tensor_mul(tmp[:], sifo[:, 0:512], tg[:])
                    nc.vector.tensor_mul(c_st[nm][:], sifo[:, 512:1024], c_st[nm][:])
                    nc.vector.tensor_add(c_st[nm][:], c_st[nm][:], tmp[:])
                    tc_sb = wk.tile([16, 512], F32, tag="tc")
                    nc.scalar.activation(tc_sb[:], c_st[nm][:], AF.Tanh)
                    h_sb = wk.tile([16, 512], F32R, tag="h")
                    nc.vector.tensor_mul(h_sb[:], sifo[:, 1024:1536], tc_sb[:])
                    tps = tpp.tile([128, 64], F32R, tag="tp")
                    for kc in range(4):
                        nc.tensor.transpose(
                            tps[:, kc * 16:(kc + 1) * 16],
                            h_sb[:, kc * 128:(kc + 1) * 128],
                            i16rep[0:16, :])
                    nc.vector.tensor_copy(hT_st[nm][:], tps[:])
                    ev = encT_dram[nm][:].rearrange("p (k t b) -> p k t b", k=4, b=BL)
                    nc.sync.dma_start(
                        ev[:, :, t, :],
                        hT_st[nm][:].rearrange("p (k b) -> p k b", k=4))

        # ---------------- Phase 3: attention + heads ----------------------
        HB = BL // 4  # quarter-batch for the qw/pre stage
        with tc.tile_pool(name="attps", bufs=4, space=bass.MemorySpace.PSUM) as aps, \
             tc.tile_pool(name="scps", bufs=2, space=bass.MemorySpace.PSUM) as scp, \
             tc.tile_pool(name="attsb", bufs=2) as asb, \
             tc.tile_pool(name="attb", bufs=1) as abp, \
             tc.tile_pool(name="wstr", bufs=3) as wsp, \
             tc.tile_pool(name="enccol", bufs=1) as ecp:

            attb = {}
            for b_ in ["batt_qn", "batt_hn", "batt_q", "batt_h", "bout_sum",
                       "bnum_sum"]:
                attb[b_] = abp.tile([128, 4], F32, tag=b_, name=f"b_{b_}")
                nc.sync.dma_start(attb[b_][:], ins[b_])
            bno_sb = abp.tile([4, 1], F32, tag="bno")
            nc.sync.dma_start(bno_sb[:], ins["bno"])
            b0_sb = abp.tile([1, 1], F32, tag="b0")
            nc.sync.dma_start(b0_sb[:], ins["b0"])
            m1 = {}
            for mn, T in [("m1q", Q), ("m1s", S)]:
                m1[mn] = abp.tile([128, BL * T], F32, tag=mn, name=f"m_{mn}")
                nc.sync.dma_start(m1[mn][:], ins[mn])

            encT_col = ecp.tile([128, 4 * C * BL], F32R, tag="encT_col")
            nc.sync.dma_start(encT_col[:], encT_dram["col"][:])
            ecv = encT_col[:].rearrange("p (k t b) -> p k t b", k=4, b=BL)
            encT = {}
            for nm, T in [("q", Q), ("hs", S)]:
                encT[nm] = ecp.tile([128, 4 * T * BL], F32R,
                                    tag=f"encT_{nm}", name=f"encTs_{nm}")
                nc.sync.dma_start(encT[nm][:], encT_dram[nm][:])
            encTv_qh = {nm: encT[nm][:].rearrange("p (k t b) -> p k t b", k=4, b=BL)
                        for nm in ("q", "hs")}

            # --- projections (consume encT_q/hs, then free them) -----------
            proj = {}
            with tc.tile_pool(name="proj", bufs=1) as prj:
                with tc.tile_pool(name="projw", bufs=1) as pwp:
                    for pname, wname, bname, src, T in [
                            ("qn", "watt_qn", "batt_qn", "q", Q),
                            ("hn", "watt_hn", "batt_hn", "hs", S),
                            ("qa", "watt_q", "batt_q", "q", Q),
                            ("ha", "watt_h", "batt_h", "hs", S)]:
                        ntok = T * BL
                        wt = pwp.tile([128, 4 * H], F32R, tag="w", name=f"wp_{pname}")
                        nc.sync.dma_start(wt[:], ins[wname])
                        pt = prj.tile([128, 4 * ntok], F32R, tag=f"proj_{pname}",
                                      name=f"proj_{pname}")
                        proj[pname] = pt
                        for jc in range(4):
                            for ch in range(ntok // 512):
                                ps = aps.tile([128, 512], F32, tag="ps1")
                                for kc in range(4):
                                    nc.tensor.matmul(
                                        ps[:],
                                        _r(wt[:, kc * H + jc * 128:
                                              kc * H + (jc + 1) * 128]),
                                        _r(encT[src][:, kc * ntok + ch * 512:
                                                     kc * ntok + (ch + 1) * 512]),
                                        start=(kc == 0), stop=(kc == 3))
                                nc.vector.tensor_scalar_add(
                                    pt[:, jc * ntok + ch * 512:
                                       jc * ntok + (ch + 1) * 512],
                                    ps[:], attb[bname][:, jc:jc + 1])
                projv = {k: v[:].rearrange("p (j t b) -> p j t b", j=4, b=BL)
                         for k, v in proj.items()}

                # --- per-sample scores/softmax/weighted sums, in halves ----
                qwnT = {"qn": asb.tile([128, 64], F32R, tag="qwnT_qn", bufs=1,
                                       name="qwnT_qn"),
                        "hn": asb.tile([128, 64], F32R, tag="qwnT_hn", bufs=1,
                                       name="qwnT_hn")}
                for half in range(4):
                    qwT = {}
                    for srcT in ("qa", "ha"):
                        qwT[srcT] = asb.tile([128, 4 * C * HB], F32R,
                                             tag=f"qwT_{srcT}", bufs=1,
                                             name=f"qwT_{srcT}_{half}")
                    for bh in range(HB):
                        b_ = half * HB + bh  # noqa
                        encb = {}
                        for src, T in [("q", Q), ("hs", S)]:
                            encb[src] = asb.tile([T, 512], F32R, tag=f"encb_{src}",
                                                 name=f"encb_{src}")
                            for kc in range(4):
                                tps = aps.tile([T, 128], F32R, tag="ps1")
                                nc.tensor.transpose(
                                    tps[:], encTv_qh[src][:, kc, :, b_], ident[:])
                                nc.vector.tensor_copy(
                                    encb[src][:, kc * 128:(kc + 1) * 128], tps[:])
                        for pname, src, T, mn in [("qn", "q", Q, "m1q"),
                                                  ("hn", "hs", S, "m1s"),
                                                  ("qa", "q", Q, "m1q"),
                                                  ("ha", "hs", S, "m1s")]:
                            ps = aps.tile([128, T], F32, tag="ps1")
                            for kc in range(4):
                                nc.tensor.matmul(
                                    ps[:], _r(ecv[:, kc, :, b_]),
                                    _r(projv[pname][:, kc, :, b_]),
                                    start=(kc == 0), stop=(kc == 3))
                            att = asb.tile([128, T], F32, tag="attsb")
                            nc.vector.tensor_scalar_add(att[:], ps[:], 100.0)
                            nc.vector.tensor_mul(
                                att[:], att[:], m1[mn][:, b_ * T:(b_ + 1) * T])
                            nmx = asb.tile([128, 1], F32, tag="nmx")
                            nc.vector.reduce_max(nmx[:], att[:], axis=AX.X,
                                                 negate=True)
                            ex = asb.tile([128, T], F32, tag="ex")
                            sm = asb.tile([128, 1], F32, tag="sm")
                            nc.scalar.activation(ex[:], att[:], AF.Exp,
                                                 bias=nmx[:], accum_out=sm[:])
                            rs = asb.tile([128, 1], F32, tag="rs")
                            nc.vector.reciprocal(rs[:], sm[:])
                            prob = asb.tile([128, T], F32R, tag="prob")
                            nc.vector.tensor_scalar_mul(prob[:], ex[:], rs[:])

                            if pname in ("qn", "hn"):
                                sps = aps.tile([T, 4], F32, tag="ps1")
                                nc.tensor.matmul(sps[:], _r(prob[:]),
                                                 _r(ones4[:]),
                                                 start=True, stop=True)
                                ssb = asb.tile([T, 4], F32R, tag="ssb")
                                nc.vector.tensor_copy(ssb[:], sps[:])
                                qn_ps = aps.tile([128, 16], F32, tag="ps1")
                                for jc in range(4):
                                    nc.tensor.matmul(
                                        qn_ps[:, jc * 4:(jc + 1) * 4],
                                        _r(encb[src][:, jc * 128:(jc + 1) * 128]),
                                        _r(ssb[:]), start=True, stop=True)
                                qnv = qwnT[pname][:].rearrange(
                                    "p (j b) -> p j b", j=4)
                                nc.vector.tensor_copy(
                                    qnv[:, :, b_],
                                    qn_ps[:].rearrange("p (j r) -> p j r",
                                                       r=4)[:, :, 0])
                            else:
                                pts = aps.tile([T, 128], F32R, tag="ps1")
                                nc.tensor.transpose(pts[:], prob[:], ident[:])
                                ptsb = asb.tile([T, 128], F32R, tag="ptsb")
                                nc.vector.tensor_copy(ptsb[:], pts[:])
                                qw_ps = aps.tile([128, 512], F32, tag="ps1")
                                for jc in range(4):
                                    nc.tensor.matmul(
                                        qw_ps[:, jc * 128:(jc + 1) * 128],
                                        _r(encb[src][:, jc * 128:(jc + 1) * 128]),
                                        _r(ptsb[:]), start=True, stop=True)
                                dstv = qwT[pname][:].rearrange(
                                    "p (k c b) -> p k c b", k=4, b=HB)
                                nc.vector.tensor_copy(
                                    dstv[:, :, :, bh],
                                    qw_ps[:].rearrange("p (k c) -> p k c", k=4))

                    # --- col-head output for this half ---------------------
                    wout = {}
                    for wname in ["wout_q", "wout_hs", "wout_c"]:
                        wout[wname] = wsp.tile([128, 4 * H], F32R, tag="w",
                                               name=f"wo_{wname}_{half}")
                        nc.sync.dma_start(wout[wname][:], ins[wname])
                    vout = wsp.tile([128, 4], F32R, tag="vout", name=f"vo_{half}")
                    nc.sync.dma_start(vout[:], ins["vout"])
                    sc_ps = scp.tile([1, 512], F32, tag="scps")
                    for j2c in range(4):
                        pre = aps.tile([128, 512], F32, tag="ps1")
                        first = True
                        for wname, srcT in [("wout_q", "qa"), ("wout_hs", "ha")]:
                            for kc in range(4):
                                nc.tensor.matmul(
                                    pre[:],
                                    _r(wout[wname][:, kc * H + j2c * 128:
                                                   kc * H + (j2c + 1) * 128]),
                                    _r(qwT[srcT][:, kc * C * HB:
                                                 (kc + 1) * C * HB]),
                                    start=first, stop=False)
                                first = False
                        for kc in range(4):
                            nc.tensor.matmul(
                                pre[:],
                                _r(wout["wout_c"][:, kc * H + j2c * 128:
                                                  kc * H + (j2c + 1) * 128]),
                                _r(ecv[:, kc, :, half * HB:(half + 1) * HB]),
                                start=False, stop=(kc == 3))
                        tpre = asb.tile([128, 512], F32R, tag="tpre")
                        nc.scalar.activation(
                            tpre[:], pre[:], AF.Tanh,
                            bias=attb["bout_sum"][:, j2c:j2c + 1])
                        nc.tensor.matmul(
                            sc_ps[:], _r(vout[:, j2c:j2c + 1]), _r(tpre[:]),
                            start=(j2c == 0), stop=(j2c == 3))
                    csb = asb.tile([1, 512], F32, tag="csb")
                    nc.vector.tensor_scalar_add(csb[:], sc_ps[:], b0_sb[:])
                    nc.sync.dma_start(
                        out_col[:, half * 512:(half + 1) * 512], csb[:])

            # --- num head --------------------------------------------------
            wnum = {}
            for wname in ["wnum_q", "wnum_hs"]:
                wnum[wname] = wsp.tile([128, 4 * H], F32R, tag="w",
                                       name=f"wn_{wname}")
                nc.sync.dma_start(wnum[wname][:], ins[wname])
            wno = wsp.tile([128, 16], F32R, tag="wno", name="wno_t")
            nc.sync.dma_start(wno[:], ins["wno"])
            zT = asb.tile([128, 64], F32R, tag="zT", bufs=1)
            for j2c in range(4):
                zps = aps.tile([128, 16], F32, tag="ps1")
                first = True
                for wname, srcn in [("wnum_q", "qn"), ("wnum_hs", "hn")]:
                    for kc in range(4):
                        nc.tensor.matmul(
                            zps[:],
                            _r(wnum[wname][:, kc * H + j2c * 128:
                                           kc * H + (j2c + 1) * 128]),
                            _r(qwnT[srcn][:, kc * 16:(kc + 1) * 16]),
                            start=first, stop=(wname == "wnum_hs" and kc == 3))
                        first = False
                nc.scalar.activation(zT[:, j2c * 16:(j2c + 1) * 16], zps[:],
                                     AF.Tanh, bias=attb["bnum_sum"][:, j2c:j2c + 1])
            z2 = aps.tile([4, 16], F32, tag="ps1")
            for kc in range(4):
                nc.tensor.matmul(z2[:], _r(wno[:, kc * 4:(kc + 1) * 4]),
                                 _r(zT[:, kc * 16:(kc + 1) * 16]),
                                 start=(kc == 0), stop=(kc == 3))
            nsb = asb.tile([4, 16], F32, tag="nsb")
            nc.vector.tensor_scalar_add(nsb[:], z2[:], bno_sb[:])
            nc.sync.dma_start(out_num, nsb[:])

    nc.compile()
    return nc


def _chunkk(a, nch, P=128):
    """[nch*P, N] -> [P, nch*N] with col = kc*N + n."""
    K, N = a.shape
    assert K == nch * P
    return np.ascontiguousarray(
        a.reshape(nch, P, N).transpose(1, 0, 2).reshape(P, nch * N))


def _prep_core(xq, xh, xc, mq, mh, mc, params):
    """Build the per-core input map (xq:[16,64,300] etc, masks [16,T])."""
    m = {}
    for nm, T, x in [("q", Q, xq), ("hs", S, xh), ("col", C, xc)]:
        ntok = T * BL
        xt = np.zeros((DP, ntok), np.float32)
        xt[:D] = x.transpose(2, 1, 0).reshape(D, ntok)  # [k, t*16+b]
        xt[D] = 1.0
        m[f"xt_{nm}"] = _chunkk(xt, 3)
        Wih, Whh, bih, bhh = [np.asarray(a, np.float32) for a in params[f"{nm}_lstm"]]
        wih = np.zeros((DP, G), np.float32)
        wih[:D] = Wih[PERM].T
        wih[D] = (bih + bhh)[PERM]
        m[f"wih_{nm}"] = _chunkk(wih, 3)
        m[f"whh_{nm}"] = _chunkk(np.ascontiguousarray(Whh[PERM].T), 4)
    m["i16rep"] = np.tile(np.eye(16, dtype=np.float32), (8, 1))

    m["ident"] = np.eye(128, dtype=np.float32)
    m["ones4"] = np.ones((128, 4), np.float32)
    mqf = mq.astype(np.float32); mhf = mh.astype(np.float32)
    mcf = mc.astype(np.float32)
    m["m1q"] = np.ascontiguousarray(
        np.einsum("bc,bq->cbq", mcf, mqf).reshape(C, BL * Q))
    m["m1s"] = np.ascontiguousarray(
        np.einsum("bc,bs->cbs", mcf, mhf).reshape(C, BL * S))
    for wname, pname in [("watt_qn", "q_num_att"), ("watt_hn", "hs_num_att"),
                         ("watt_q", "q_att"), ("watt_h", "hs_att"),
                         ("wout_q", "col_out_q"), ("wout_hs", "col_out_hs"),
                         ("wout_c", "col_out_c"),
                         ("wnum_q", "col_num_out_q"), ("wnum_hs", "col_num_out_hs")]:
        W = np.asarray(params[pname][0], np.float32)
        m[wname] = _chunkk(np.ascontiguousarray(W.T), 4)
    for bname, pnames in [("batt_qn", ["q_num_att"]), ("batt_hn", ["hs_num_att"]),
                          ("batt_q", ["q_att"]), ("batt_h", ["hs_att"]),
                          ("bout_sum", ["col_out_q", "col_out_hs", "col_out_c"]),
                          ("bnum_sum", ["col_num_out_q", "col_num_out_hs"])]:
        bsum = sum(np.asarray(params[p][1], np.float32) for p in pnames)
        m[bname] = np.ascontiguousarray(bsum.reshape(4, 128).T)
    Wno, bno = [np.asarray(a, np.float32) for a in params["col_num_out"]]
    m["wno"] = _chunkk(np.ascontiguousarray(Wno.T), 4)   # [512,4] -> [128, 16]
    m["bno"] = bno.reshape(4, 1)
    Wv, b0 = [np.asarray(a, np.float32) for a in params["col_out"]]
    m["vout"] = _chunkk(np.ascontiguousarray(Wv.T), 4)   # [512,1] -> [128,4]
    m["b0"] = np.array([[b0[0]]], np.float32)
    return m


_CACHED = {}


def kernel(q_emb_var, q_len, q_recover, hs_emb_var, hs_len, hs_recover,
           col_emb_var, col_len, col_recover, params):
    qe = np.asarray(q_emb_var, np.float32)[np.asarray(q_recover)]
    he = np.asarray(hs_emb_var, np.float32)[np.asarray(hs_recover)]
    ce = np.asarray(col_emb_var, np.float32)[np.asarray(col_recover)]
    qlen = np.asarray(q_len)[np.asarray(q_recover)]
    hlen = np.asarray(hs_len)[np.asarray(hs_recover)]
    clen = np.asarray(col_len)[np.asarray(col_recover)]
    mq = (np.arange(Q)[None, :] < qlen[:, None])
    mh = (np.arange(S)[None, :] < hlen[:, None])
    mc = (np.arange(C)[None, :] < clen[:, None])

    if "nc" not in _CACHED:
        _CACHED["nc"] = build_program()
    nc = _CACHED["nc"]

    in_maps = []
    for i in range(NC):
        sl = slice(i * BL, (i + 1) * BL)
        in_maps.append(_prep_core(qe[sl], he[sl], ce[sl], mq[sl], mh[sl], mc[sl],
                                  params))
    trace = bool(int(os.environ.get("KERNEL_TRACE", "0")))
    r = run_bass_kernel_spmd(nc, in_maps, list(range(NC)), trace=trace)
    _CACHED["exec_time_ns"] = r.exec_time_ns
    res = r.results

    col_num = np.zeros((B, NL), np.float32)
    col_sc = np.zeros((B, C), np.float32)
    for i in range(NC):
        sl = slice(i * BL, (i + 1) * BL)
        col_num[sl] = res[i]["out_num"].T
        oc = res[i]["out_col"].reshape(4, C, 4)   # [qtr, c, bq]
        col_sc[sl] = oc.transpose(0, 2, 1).reshape(BL, C)
    return col_num, col_sc


# revision 25
# speedup vs baseline: 1.8458x; 1.8458x over previous
"""Trainium2 Bass kernel for nn_ColPredictor (3x LSTM + dual attention heads).

Self-contained: hardcoded shapes, pure data-parallel over batch (B=128 -> 16/core
on 8 cores). Per core: fused-bias GEMM for LSTM input transforms (xg), a
tri-LSTM-interleaved recurrence (float32r matmuls, PE transposes for the hidden
state), then attention + output heads. Outputs are returned in host-friendly
layouts and reassembled in numpy.
"""

import os
import sys
import numpy as np
from contextlib import ExitStack

sys.path.insert(0, "/opt/trn_rl_repo")

import concourse.bass as bass
import concourse.tile as tile
from concourse import bacc, mybir
from concourse.bass_utils import run_bass_kernel_spmd

F32 = mybir.dt.float32
F32R = mybir.dt.float32r
AF = mybir.ActivationFunctionType
AX = mybir.AxisListType

B, Q, S, C, D, H = 128, 64, 32, 128, 300, 512
G = 4 * H          # 2048 gate width
NC = 8             # cores
BL = B // NC       # 16 batch per core
DP = 384           # padded contraction for x (300 data + bias row at 300)
NL = 4             # num labels
# gate reorder: torch (i,f,g,o) -> (i,f,o,g) so sigmoid gates are contiguous
PERM = np.r_[0:1024, 1536:2048, 1024:1536]

LSTMS = [("col", C), ("q", Q), ("hs", S)]   # descending seqlen; partitions 0/32/64
TMAX = C


def _r(x32r):
    return x32r.bitcast(F32R)


def build_program():
    nc = bacc.Bacc("TRN2", target_bir_lowering=False, debug=False, num_devices=NC)

    def inp(name, shape, dt=F32R):
        return nc.dram_tensor(name, shape, dt, kind="ExternalInput").ap()

    ins = {}
    for nm, T in LSTMS:
        ins[f"xt_{nm}"] = inp(f"xt_{nm}", [128, 3 * T * BL])
        ins[f"wih_{nm}"] = inp(f"wih_{nm}", [128, 3 * G])
        ins[f"whh_{nm}"] = inp(f"whh_{nm}", [128, 4 * G])
    ins["i16rep"] = inp("i16rep", [128, 16])       # eye(16) tiled 8x along partitions

    ins["ident"] = inp("ident", [128, 128])        # eye(128)
    ins["ones4"] = inp("ones4", [128, 4])
    ins["m1q"] = inp("m1q", [128, BL * Q], F32)         # outer(col_mask, q_mask) per b
    ins["m1s"] = inp("m1s", [128, BL * S], F32)
    for w in ["watt_qn", "watt_hn", "watt_q", "watt_h",
              "wout_q", "wout_hs", "wnum_q", "wnum_hs", "wout_c"]:
        ins[w] = inp(w, [128, 4 * H])
    for b_ in ["batt_qn", "batt_hn", "batt_q", "batt_h", "bout_sum", "bnum_sum"]:
        ins[b_] = inp(b_, [128, 4], F32)
    ins["wno"] = inp("wno", [128, 16])             # col_num_out.T chunks [128, kc*4+l]
    ins["bno"] = inp("bno", [4, 1], F32)
    ins["vout"] = inp("vout", [128, 4])            # col_out.T chunks
    ins["b0"] = inp("b0", [1, 1], F32)

    out_num = nc.dram_tensor("out_num", [NL, BL], F32, kind="ExternalOutput").ap()
    out_col = nc.dram_tensor("out_col", [1, BL * C], F32, kind="ExternalOutput").ap()

    # internal DRAM for xg and encodings (transposed layout)
    xg_dram = {nm: nc.dram_tensor(f"xg_{nm}", [T * BL, G], F32R).ap() for nm, T in LSTMS}
    encT_dram = {nm: nc.dram_tensor(f"encT_{nm}", [128, 4 * T * BL], F32R).ap()
                 for nm, T in LSTMS}

    with tile.TileContext(nc) as tc, ExitStack() as top:
        cpool = top.enter_context(tc.tile_pool(name="consts", bufs=1))

        def load_const(name, shape):
            t = cpool.tile(shape, F32R, tag=name, name=name)
            nc.sync.dma_start(t[:], ins[name])
            return t

        i16rep = load_const("i16rep", [128, 16])
        ident = load_const("ident", [128, 128])
        ones4 = load_const("ones4", [128, 4])

        # ---------------- Phase 1: xg = [x;1] @ [Wih; bias].T  ------------
        with tc.tile_pool(name="p1", bufs=2) as p1, \
             tc.tile_pool(name="p1psum", bufs=2, space=bass.MemorySpace.PSUM) as p1p, \
             tc.tile_pool(name="p1out", bufs=3) as p1o:
            for nm, T in LSTMS:
                ntok = T * BL
                wih = p1.tile([128, 3 * G], F32R, tag="wih")
                nc.sync.dma_start(wih[:], ins[f"wih_{nm}"])
                xt = p1.tile([128, 3 * ntok], F32R, tag="xt")
                nc.sync.dma_start(xt[:], ins[f"xt_{nm}"])
                for tc128 in range(ntok // 128):
                    for n4 in range(4):
                        ps = p1p.tile([128, 512], F32, tag="xgp")
                        for kc in range(3):
                            nc.tensor.matmul(
                                ps[:],
                                _r(xt[:, kc * ntok + tc128 * 128:
                                      kc * ntok + (tc128 + 1) * 128]),
                                _r(wih[:, kc * G + n4 * 512: kc * G + (n4 + 1) * 512]),
                                start=(kc == 0), stop=(kc == 2))
                        ev = p1o.tile([128, 512], F32R, tag="xgev")
                        nc.vector.tensor_copy(ev[:], ps[:])
                        nc.sync.dma_start(
                            xg_dram[nm][tc128 * 128:(tc128 + 1) * 128,
                                        n4 * 512:(n4 + 1) * 512], ev[:])

        # ---------------- Phase 2: recurrence ----------------------------
        with tc.tile_pool(name="whh", bufs=1) as whp, \
             tc.tile_pool(name="rec", bufs=1) as rec, \
             tc.tile_pool(name="xg", bufs=6) as xgp, \
             tc.tile_pool(name="ht", bufs=6) as htp, \
             tc.tile_pool(name="recps", bufs=4, space=bass.MemorySpace.PSUM) as rpp, \
             tc.tile_pool(name="trps", bufs=2, space=bass.MemorySpace.PSUM) as tpp, \
             tc.tile_pool(name="work", bufs=3) as wk:
            whh = {}
            for nm, T in LSTMS:
                whh[nm] = whp.tile([128, 4 * G], F32R, tag=f"whh_{nm}", name=f"whh_{nm}")
                nc.sync.dma_start(whh[nm][:], ins[f"whh_{nm}"])

            c_st = {}
            hT_st = {}
            for nm, _ in LSTMS:
                c_st[nm] = rec.tile([16, 512], F32, tag=f"c_{nm}", name=f"c_{nm}")
                nc.gpsimd.memset(c_st[nm][:], 0.0)
                hT_st[nm] = rec.tile([128, 64], F32R, tag=f"hTs_{nm}",
                                     name=f"hTs_{nm}")

            for t in range(TMAX):
                active = [(i, nm, T) for i, (nm, T) in enumerate(LSTMS) if t < T]
                for li, nm, T in active:
                    xg_sb = xgp.tile([16, G], F32R, tag="xg")
                    nc.sync.dma_start(xg_sb[:], xg_dram[nm][t * BL:(t + 1) * BL, :])
                    sifo = wk.tile([16, 1536], F32, tag="sifo")
                    tg = wk.tile([16, 512], F32, tag="tg")
                    for n4 in range(4):
                        ps_n = rpp.tile([16, 512], F32, tag="psum2")
                        nc.tensor.matmul(
                            ps_n[:], _r(i16rep[0:16, :]),
                            _r(xg_sb[:, n4 * 512:(n4 + 1) * 512]),
                            start=True, stop=(t == 0))
                        if t > 0:
                            hv = hT_st[nm][:].rearrange("p (k b) -> p k b", k=4)
                            for kc in range(4):
                                nc.tensor.matmul(
                                    ps_n[:], _r(hv[:, kc, :]),
                                    _r(whh[nm][:, kc * G + n4 * 512:
                                               kc * G + (n4 + 1) * 512]),
                                    start=False, stop=(kc == 3))
                        if n4 < 3:
                            nc.scalar.activation(
                                sifo[:, n4 * 512:(n4 + 1) * 512], ps_n[:],
                                AF.Sigmoid)
                        else:
                            nc.scalar.activation(tg[:], ps_n[:], AF.Tanh)
                    tmp = wk.tile([16, 512], F32, tag="tmp")
                    nc.vector.tensor_mul(tmp[:], sifo[:, 0:512], tg[:])
                    nc.vector.tensor_mul(c_st[nm][:], sifo[:, 512:1024], c_st[nm][:])
                    nc.vector.tensor_add(c_st[nm][:], c_st[nm][:], tmp[:])
                    tc_sb = wk.tile([16, 512], F32, tag="tc")
                    nc.scalar.activation(tc_sb[:], c_st[nm][:], AF.Tanh)
                    h_sb = wk.tile([16, 512], F32R, tag="h")
                    nc.vector.tensor_mul(h_sb[:], sifo[:, 1024:1536], tc_sb[:])
                    tps = tpp.tile([128, 64], F32R, tag="tp")
                    for kc in range(4):
                        nc.tensor.transpose(
                            tps[:, kc * 16:(kc + 1) * 16],
                            h_sb[:, kc * 128:(kc + 1) * 128],
                            i16rep[0:16, :])
                    nc.vector.tensor_copy(hT_st[nm][:], tps[:])
                    ev = encT_dram[nm][:].rearrange("p (k t b) -> p k t b", k=4, b=BL)
                    nc.sync.dma_start(
                        ev[:, :, t, :],
                        hT_st[nm][:].rearrange("p (k b) -> p k b", k=4))

        # ---------------- Phase 3: attention + heads ----------------------
        HB = BL // 4  # quarter-batch for the qw/pre stage
        with tc.tile_pool(name="attps", bufs=4, space=bass.MemorySpace.PSUM) as aps, \
             tc.tile_pool(name="scps", bufs=2, space=bass.MemorySpace.PSUM) as scp, \
             tc.tile_pool(name="attsb", bufs=2) as asb, \
             tc.tile_pool(name="attb", bufs=1) as abp, \
             tc.tile_pool(name="wstr", bufs=3) as wsp, \
             tc.tile_pool(name="enccol", bufs=1) as ecp:

            attb = {}
            for b_ in ["batt_qn", "batt_hn", "batt_q", "batt_h", "bout_sum",
                       "bnum_sum"]:
                attb[b_] = abp.tile([128, 4], F32, tag=b_, name=f"b_{b_}")
                nc.sync.dma_start(attb[b_][:], ins[b_])
            bno_sb = abp.tile([4, 1], F32, tag="bno")
            nc.sync.dma_start(bno_sb[:], ins["bno"])
            b0_sb = abp.tile([1, 1], F32, tag="b0")
            nc.sync.dma_start(b0_sb[:], ins["b0"])
            m1 = {}
            for mn, T in [("m1q", Q), ("m1s", S)]:
                m1[mn] = abp.tile([128, BL * T], F32, tag=mn, name=f"m_{mn}")
                nc.sync.dma_start(m1[mn][:], ins[mn])

            encT_col = ecp.tile([128, 4 * C * BL], F32R, tag="encT_col")
            nc.sync.dma_start(encT_col[:], encT_dram["col"][:])
            ecv = encT_col[:].rearrange("p (k t b) -> p k t b", k=4, b=BL)
            encT = {}
            for nm, T in [("q", Q), ("hs", S)]:
                encT[nm] = ecp.tile([128, 4 * T * BL], F32R,
                                    tag=f"encT_{nm}", name=f"encTs_{nm}")
                nc.sync.dma_start(encT[nm][:], encT_dram[nm][:])
            encTv_qh = {nm: encT[nm][:].rearrange("p (k t b) -> p k t b", k=4, b=BL)
                        for nm in ("q", "hs")}

            # --- projections (consume encT_q/hs, then free them) -----------
            proj = {}
            with tc.tile_pool(name="proj", bufs=1) as prj:
                with tc.tile_pool(name="projw", bufs=1) as pwp:
                    for pname, wname, bname, src, T in [
                            ("qn", "watt_qn", "batt_qn", "q", Q),
                            ("hn", "watt_hn", "batt_hn", "hs", S),
                            ("qa", "watt_q", "batt_q", "q", Q),
                            ("ha", "watt_h", "batt_h", "hs", S)]:
                        ntok = T * BL
                        wt = pwp.tile([128, 4 * H], F32R, tag="w", name=f"wp_{pname}")
                        nc.sync.dma_start(wt[:], ins[wname])
                        pt = prj.tile([128, 4 * ntok], F32R, tag=f"proj_{pname}",
                                      name=f"proj_{pname}")
                        proj[pname] = pt
                        for jc in range(4):
                            for ch in range(ntok // 512):
                                ps = aps.tile([128, 512], F32, tag="ps1")
                                for kc in range(4):
                                    nc.tensor.matmul(
                                        ps[:],
                                        _r(wt[:, kc * H + jc * 128:
                                              kc * H + (jc + 1) * 128]),
                                        _r(encT[src][:, kc * ntok + ch * 512:
                                                     kc * ntok + (ch + 1) * 512]),
                                        start=(kc == 0), stop=(kc == 3))
                                nc.vector.tensor_scalar_add(
                                    pt[:, jc * ntok + ch * 512:
                                       jc * ntok + (ch + 1) * 512],
                                    ps[:], attb[bname][:, jc:jc + 1])
                projv = {k: v[:].rearrange("p (j t b) -> p j t b", j=4, b=BL)
                         for k, v in proj.items()}

                # --- per-sample scores/softmax/weighted sums, in halves ----
                qwnT = {"qn": asb.tile([128, 64], F32R, tag="qwnT_qn", bufs=1,
                                       name="qwnT_qn"),
                        "hn": asb.tile([128, 64], F32R, tag="qwnT_hn", bufs=1,
                                       name="qwnT_hn")}
                for half in range(4):
                    qwT = {}
                    for srcT in ("qa", "ha"):
                        qwT[srcT] = asb.tile([128, 4 * C * HB], F32R,
                                             tag=f"qwT_{srcT}", bufs=1,
                                             name=f"qwT_{srcT}_{half}")
                    for bh in range(HB):
                        b_ = half * HB + bh  # noqa
                        encb = {}
                        for src, T in [("q", Q), ("hs", S)]:
                            encb[src] = asb.tile([T, 512], F32R, tag=f"encb_{src}",
                                                 name=f"encb_{src}")
                            for kc in range(4):
                                tps = aps.tile([T, 128], F32R, tag="ps1")
                                nc.tensor.transpose(
                                    tps[:], encTv_qh[src][:, kc, :, b_], ident[:])
                                nc.vector.tensor_copy(
                                    encb[src][:, kc * 128:(kc + 1) * 128], tps[:])
                        for pname, src, T, mn in [("qn", "q", Q, "m1q"),
                                                  ("hn", "hs", S, "m1s"),
                                                  ("qa", "q", Q, "m1q"),
                                                  ("ha", "hs", S, "m1s")]:
                            ps = aps.tile([128, T], F32, tag="ps1")
                            for kc in range(4):
                                nc.tensor.matmul(
                                    ps[:], _r(ecv[:, kc, :, b_]),
                                    _r(projv[pname][:, kc, :, b_]),
                                    start=(kc == 0), stop=(kc == 3))
                            att = asb.tile([128, T], F32, tag="attsb")
                            nc.vector.tensor_scalar_add(att[:], ps[:], 100.0)
                            nc.vector.tensor_mul(
                                att[:], att[:], m1[mn][:, b_ * T:(b_ + 1) * T])
                            nmx = asb.tile([128, 1], F32, tag="nmx")
                            nc.vector.reduce_max(nmx[:], att[:], axis=AX.X,
                                                 negate=True)
                            ex = asb.tile([128, T], F32, tag="ex")
                            sm = asb.tile([128, 1], F32, tag="sm")
                            nc.scalar.activation(ex[:], att[:], AF.Exp,
                                                 bias=nmx[:], accum_out=sm[:])
                            rs = asb.tile([128, 1], F32, tag="rs")
                            nc.vector.reciprocal(rs[:], sm[:])
                            prob = asb.tile([128, T], F32R, tag="prob")
                            nc.vector.tensor_scalar_mul(prob[:], ex[:], rs[:])

                            if pname in ("qn", "hn"):
                                sps = aps.tile([T, 4], F32, tag="ps1")
                                nc.tensor.matmul(sps[:], _r(prob[:]),
                                                 _r(ones4[:]),
                                                 start=True, stop=True)
                                ssb = asb.tile([T, 4], F32R, tag="ssb")
                                nc.vector.tensor_copy(ssb[:], sps[:])
                                qn_ps = aps.tile([128, 16], F32, tag="ps1")
                                for jc in range(4):
                                    nc.tensor.matmul(
                                        qn_ps[:, jc * 4:(jc + 1) * 4],
                                        _r(encb[src][:, jc * 128:(jc + 1) * 128]),
                                        _r(ssb[:]), start=True, stop=True)
                                qnv = qwnT[pname][:].rearrange(
                                    "p (j b) -> p j b", j=4)
                                nc.vector.tensor_copy(
                                    qnv[:, :, b_],
                                    qn_ps[:].rearrange("p (j r) -> p j r",
                                                       r=4)[:, :, 0])
                            else:
                                pts = aps.tile([T, 128], F32R, tag="ps1")
                                nc.tensor.transpose(pts[:], prob[:], ident[:])
                                ptsb = asb.tile([T, 128], F32R, tag="ptsb")
                                nc.vector.tensor_copy(ptsb[:], pts[:])
                                qw_ps = aps.tile([128, 512], F32, tag="ps1")
                                for jc in range(4):
                                    nc.tensor.matmul(
                                        qw_ps[:, jc * 128:(jc + 1) * 128],
                                        _r(encb[src][:, jc * 128:(jc + 1) * 128]),
                                        _r(ptsb[:]), start=True, stop=True)
                                dstv = qwT[pname][:].rearrange(
                                    "p (k c b) -> p k c b", k=4, b=HB)
                                nc.vector.tensor_copy(
                                    dstv[:, :, :, bh],
                                    qw_ps[:].rearrange("p (k c) -> p k c", k=4))

                    # --- col-head output for this half ---------------------
                    wout = {}
                    for wname in ["wout_q", "wout_hs", "wout_c"]:
                        wout[wname] = wsp.tile([128, 4 * H], F32R, tag="w",
                                               name=f"wo_{wname}_{half}")
                        nc.sync.dma_start(wout[wname][:], ins[wname])
                    vout = wsp.tile([128, 4], F32R, tag="vout", name=f"vo_{half}")
                    nc.sync.dma_start(vout[:], ins["vout"])
                    sc_ps = scp.tile([1, 512], F32, tag="scps")
                    for j2c in range(4):
                        pre = aps.tile([128, 512], F32, tag="ps1")
                        first = True
                        for wname, srcT in [("wout_q", "qa"), ("wout_hs", "ha")]:
                            for kc in range(4):
                                nc.tensor.matmul(
                                    pre[:],
                                    _r(wout[wname][:, kc * H + j2c * 128:
                                                   kc * H + (j2c + 1) * 128]),
                                    _r(qwT[srcT][:, kc * C * HB:
                                                 (kc + 1) * C * HB]),
                                    start=first, stop=False)
                                first = False
                        for kc in range(4):
                            nc.tensor.matmul(
                                pre[:],
                                _r(wout["wout_c"][:, kc * H + j2c * 128:
                                                  kc * H + (j2c + 1) * 128]),
                                _r(ecv[:, kc, :, half * HB:(half + 1) * HB]),
                                start=False, stop=(kc == 3))
                        tpre = asb.tile([128, 512], F32R, tag="tpre")
                        nc.scalar.activation(
                            tpre[:], pre[:], AF.Tanh,
                            bias=attb["bout_sum"][:, j2c:j2c + 1])
                        nc.tensor.matmul(
                            sc_ps[:], _r(vout[:, j2c:j2c + 1]), _r(tpre[:]),
                            start=(j2c == 0), stop=(j2c == 3))
                    csb = asb.tile([1, 512], F32, tag="csb")
                    nc.vector.tensor_scalar_add(csb[:], sc_ps[:], b0_sb[:])
                    nc.sync.dma_start(
                        out_col[:, half * 512:(half + 1) * 512], csb[:])

            # --- num head --------------------------------------------------
            wnum = {}
            for wname in ["wnum_q", "wnum_hs"]:
                wnum[wname] = wsp.tile([128, 4 * H], F32R, tag="w",
                                       name=f"wn_{wname}")
                nc.sync.dma_start(wnum[wname][:], ins[wname])
            wno = wsp.tile([128, 16], F32R, tag="wno", name="wno_t")
            nc.sync.dma_start(wno[:], ins["wno"])
            zT = asb.tile([128, 64], F32R, tag="zT", bufs=1)
            for j2c in range(4):
                zps = aps.tile([128, 16], F32, tag="ps1")
                first = True
                for wname, srcn in [("wnum_q", "qn"), ("wnum_hs", "hn")]:
                    for kc in range(4):
                        nc.tensor.matmul(
                            zps[:],
                            _r(wnum[wname][:, kc * H + j2c * 128:
                                           kc * H + (j2c + 1) * 128]),
                            _r(qwnT[srcn][:, kc * 16:(kc + 1) * 16]),
                            start=first, stop=(wname == "wnum_hs" and kc == 3))
                        first = False
                nc.scalar.activation(zT[:, j2c * 16:(j2c + 1) * 16], zps[:],
                                     AF.Tanh, bias=attb["bnum_sum"][:, j2c:j2c + 1])
            z2 = aps.tile([4, 16], F32, tag="ps1")
            for kc in range(4):
                nc.tensor.matmul(z2[:], _r(wno[:, kc * 4:(kc + 1) * 4]),
                                 _r(zT[:, kc * 16:(kc + 1) * 16]),
                                 start=(kc == 0), stop=(kc == 3))
            nsb = asb.tile([4, 16], F32, tag="nsb")
            nc.vector.tensor_scalar_add(nsb[:], z2[:], bno_sb[:])
            nc.sync.dma_start(out_num, nsb[:])

    nc.compile()
    return nc


def _chunkk(a, nch, P=128):
    """[nch*P, N] -> [P, nch*N] with col = kc*N + n."""
    K, N = a.shape
    assert K == nch * P
    return np.ascontiguousarray(
        a.reshape(nch, P, N).transpose(1, 0, 2).reshape(P, nch * N))


def _prep_core(xq, xh, xc, mq, mh, mc, params):
    """Build the per-core input map (xq:[16,64,300] etc, masks [16,T])."""
    m = {}
    for nm, T, x in [("q", Q, xq), ("hs", S, xh), ("col", C, xc)]:
        ntok = T * BL
        xt = np.zeros((DP, ntok), np.float32)
        xt[:D] = x.transpose(2, 1, 0).reshape(D, ntok)  # [k, t*16+b]
        xt[D] = 1.0
        m[f"xt_{nm}"] = _chunkk(xt, 3)
        Wih, Whh, bih, bhh = [np.asarray(a, np.float32) for a in params[f"{nm}_lstm"]]
        wih = np.zeros((DP, G), np.float32)
        wih[:D] = Wih[PERM].T
        wih[D] = (bih + bhh)[PERM]
        m[f"wih_{nm}"] = _chunkk(wih, 3)
        m[f"whh_{nm}"] = _chunkk(np.ascontiguousarray(Whh[PERM].T), 4)
    m["i16rep"] = np.tile(np.eye(16, dtype=np.float32), (8, 1))

    m["ident"] = np.eye(128, dtype=np.float32)
    m["ones4"] = np.ones((128, 4), np.float32)
    mqf = mq.astype(np.float32); mhf = mh.astype(np.float32)
    mcf = mc.astype(np.float32)
    m["m1q"] = np.ascontiguousarray(
        np.einsum("bc,bq->cbq", mcf, mqf).reshape(C, BL * Q))
    m["m1s"] = np.ascontiguousarray(
        np.einsum("bc,bs->cbs", mcf, mhf).reshape(C, BL * S))
    for wname, pname in [("watt_qn", "q_num_att"), ("watt_hn", "hs_num_att"),
                         ("watt_q", "q_att"), ("watt_h", "hs_att"),
                         ("wout_q", "col_out_q"), ("wout_hs", "col_out_hs"),
                         ("wout_c", "col_out_c"),
                         ("wnum_q", "col_num_out_q"), ("wnum_hs", "col_num_out_hs")]:
        W = np.asarray(params[pname][0], np.float32)
        m[wname] = _chunkk(np.ascontiguousarray(W.T), 4)
    for bname, pnames in [("batt_qn", ["q_num_att"]), ("batt_hn", ["hs_num_att"]),
                          ("batt_q", ["q_att"]), ("batt_h", ["hs_att"]),
                          ("bout_sum", ["col_out_q", "col_out_hs", "col_out_c"]),
                          ("bnum_sum", ["col_num_out_q", "col_num_out_hs"])]:
        bsum = sum(np.asarray(params[p][1], np.float32) for p in pnames)
        m[bname] = np.ascontiguousarray(bsum.reshape(4, 128).T)
    Wno, bno = [np.asarray(a, np.float32) for a in params["col_num_out"]]
    m["wno"] = _chunkk(np.ascontiguousarray(Wno.T), 4)   # [512,4] -> [128, 16]
    m["bno"] = bno.reshape(4, 1)
    Wv, b0 = [np.asarray(a, np.float32) for a in params["col_out"]]
    m["vout"] = _chunkk(np.ascontiguousarray(Wv.T), 4)   # [512,1] -> [128,4]
    m["b0"] = np.array([[b0[0]]], np.float32)
    return m


_CACHED = {}


def kernel(q_emb_var, q_len, q_recover, hs_emb_var, hs_len, hs_recover,
           col_emb_var, col_len, col_recover, params):
    qe = np.asarray(q_emb_var, np.float32)[np.asarray(q_recover)]
    he = np.asarray(hs_emb_var, np.float32)[np.asarray(hs_recover)]
    ce = np.asarray(col_emb_var, np.float32)[np.asarray(col_recover)]
    qlen = np.asarray(q_len)[np.asarray(q_recover)]
    hlen = np.asarray(hs_len)[np.asarray(hs_recover)]
    clen = np.asarray(col_len)[np.asarray(col_recover)]
    mq = (np.arange(Q)[None, :] < qlen[:, None])
    mh = (np.arange(S)[None, :] < hlen[:, None])
    mc = (np.arange(C)[None, :] < clen[:, None])

    if "nc" not in _CACHED:
        _CACHED["nc"] = build_program()
    nc = _CACHED["nc"]

    in_maps = []
    for i in range(NC):
        sl = slice(i * BL, (i + 1) * BL)
        in_maps.append(_prep_core(qe[sl], he[sl], ce[sl], mq[sl], mh[sl], mc[sl],
                                  params))
    trace = bool(int(os.environ.get("KERNEL_TRACE", "0")))
    r = run_bass_kernel_spmd(nc, in_maps, list(range(NC)), trace=trace)
    _CACHED["exec_time_ns"] = r.exec_time_ns
    res = r.results

    col_num = np.zeros((B, NL), np.float32)
    col_sc = np.zeros((B, C), np.float32)
    for i in range(NC):
        sl = slice(i * BL, (i + 1) * BL)
        col_num[sl] = res[i]["out_num"].T
        oc = res[i]["out_col"].reshape(4, C, 4)   # [qtr, c, bq]
        col_sc[sl] = oc.transpose(0, 2, 1).reshape(BL, C)
    return col_num, col_sc
